# revision 3
# baseline (speedup 1.0000x reference)
"""Cumulative mean along T (running mean) for input [8, 4096, 1024] f32.

out[b, t, f] = mean(x[b, :t+1, f])

Pure data parallel over batch: 8 cores, one batch element each.
Per core, blocked prefix-sum along T in 128-row blocks with a rotated-output
triangular matmul:

  stationary lt[s, t]: col 0 = all ones; col t>=1 has ones for s <= t-1.
  With the running carry added into row 0 of the block beforehand,
    psum[0]    = carry + block total      (= output row 127 of the block)
    psum[t>=1] = carry + prefix(t-1)      (= output row t-1 of the block)
  so the next block's carry is read from PSUM partition 0 (legal AP base),
  and the output rows are just rotated by one partition, which the output
  DMA undoes. Per-row 1/(t+1) scaling via tensor_scalar with a per-partition
  reciprocal column (also rotated). All math in exact fp32.
"""

import numpy as np

import concourse.bacc as bacc
import concourse.tile as tile
from concourse import mybir
from concourse.bass_utils import run_bass_kernel_spmd

B, T, F = 8, 4096, 1024
P = 128
NBLK = T // P  # 32
FH = 512       # one PSUM bank of f32
NHALF = F // FH

F32 = mybir.dt.float32


def _build():
    nc = bacc.Bacc(None, target_bir_lowering=False)
    x_dram = nc.dram_tensor("x", [T, F], F32, kind="ExternalInput")
    out_dram = nc.dram_tensor("out", [T, F], F32, kind="ExternalOutput")

    # Rotated prefix-sum stationary: col 0 all ones, col t>=1 strict-upper.
    lt_np = np.triu(np.ones((P, P), dtype=np.float32), 1)
    lt_np[:, 0] = 1.0
    # recip[p, i]: psum partition p of block i holds output row
    #   r = i*128 + 127   (p == 0)
    #   r = i*128 + p - 1  (p >= 1)
    # scale is 1 / (r + 1).
    rows = np.arange(T, dtype=np.int64).reshape(NBLK, P)  # [i, p'] -> i*128+p'
    r_of_p = np.empty((NBLK, P), dtype=np.int64)
    r_of_p[:, 0] = rows[:, P - 1]
    r_of_p[:, 1:] = rows[:, : P - 1]
    recip_np = np.ascontiguousarray(
        (1.0 / (r_of_p.T + 1.0)).astype(np.float32)
    )  # [P, NBLK]
    lt_dram = nc.inline_tensor(lt_np, "lt_const")
    recip_dram = nc.inline_tensor(recip_np, "recip_const")

    with tile.TileContext(nc) as tc:
        with (
            tc.tile_pool(name="const", bufs=1) as cpool,
            tc.tile_pool(name="xin", bufs=4) as xpool,
            tc.tile_pool(name="xout", bufs=4) as opool,
            tc.tile_pool(name="run", bufs=2) as rpool,
            tc.tile_pool(name="psum", bufs=3, space="PSUM") as ppool,
        ):
            lt = cpool.tile([P, P], F32)
            nc.sync.dma_start(lt[:], lt_dram[:])
            recip = cpool.tile([P, NBLK], F32)
            nc.sync.dma_start(recip[:], recip_dram[:])

            running = None
            for i in range(NBLK):
                xt = xpool.tile([P, F], F32)
                nc.sync.dma_start(xt[:], x_dram[i * P : (i + 1) * P, :])

                if running is not None:
                    nc.vector.tensor_tensor(
                        xt[0:1, :], xt[0:1, :], running[:], mybir.AluOpType.add
                    )

                ps = ppool.tile([P, F], F32)
                for h in range(NHALF):
                    nc.tensor.matmul(
                        ps[:, h * FH : (h + 1) * FH],
                        lt[:],
                        xt[:, h * FH : (h + 1) * FH],
                        start=True,
                        stop=True,
                    )

                if i < NBLK - 1:
                    new_running = rpool.tile([1, F], F32)
                    nc.vector.tensor_copy(new_running[:], ps[0:1, :])
                    running = new_running

                ot = opool.tile([P, F], F32)
                nc.vector.tensor_scalar(
                    ot[:], ps[:], recip[:, i : i + 1], None, mybir.AluOpType.mult
                )
                # Undo the rotation: partition 0 is the block's last row.
                nc.sync.dma_start(
                    out_dram[i * P : (i + 1) * P - 1, :], ot[1:P, :]
                )
                nc.sync.dma_start(
                    out_dram[(i + 1) * P - 1 : (i + 1) * P, :], ot[0:1, :]
                )
                running = running  # keep name for clarity

    nc.compile()
    return nc


_NC_CACHE = None
last_results = None  # BassKernelResults of the most recent run (for test harness)


def kernel(inputs: np.ndarray) -> np.ndarray:
    global _NC_CACHE, last_results
    if _NC_CACHE is None:
        _NC_CACHE = _build()
    nc = _NC_CACHE
    x = np.ascontiguousarray(np.asarray(inputs, dtype=np.float32))
    assert x.shape == (B, T, F), x.shape
    in_maps = [{"x": x[b]} for b in range(B)]
    res = run_bass_kernel_spmd(nc, in_maps, core_ids=list(range(B)))
    last_results = res
    return np.stack([r["out"] for r in res.results], axis=0)


# revision 4
# speedup vs baseline: 1.0020x; 1.0020x over previous
"""Cumulative mean along T (running mean) for input [8, 4096, 1024] f32.

out[b, t, f] = mean(x[b, :t+1, f])

Pure data parallel over batch: 8 cores, one batch element each.
Per core, blocked prefix-sum along T in 128-row blocks with a rotated-output
triangular matmul:

  stationary lt[s, t]: col 0 = all ones; col t>=1 has ones for s <= t-1.
  With the running carry added into row 0 of the block beforehand,
    psum[0]    = carry + block total      (= output row 127 of the block)
    psum[t>=1] = carry + prefix(t-1)      (= output row t-1 of the block)
  so the next block's carry is read from PSUM partition 0 (legal AP base),
  and the output rows are just rotated by one partition, which the output
  DMA undoes. Per-row 1/(t+1) scaling via tensor_scalar with a per-partition
  reciprocal column (also rotated). All math in exact fp32.
"""

import numpy as np

import concourse.bacc as bacc
import concourse.tile as tile
from concourse import mybir
from concourse.bass_utils import run_bass_kernel_spmd

B, T, F = 8, 4096, 1024
P = 128
NBLK = T // P  # 32
FH = 512       # one PSUM bank of f32
NHALF = F // FH

F32 = mybir.dt.float32


def _build():
    nc = bacc.Bacc(None, target_bir_lowering=False)
    x_dram = nc.dram_tensor("x", [T, F], F32, kind="ExternalInput")
    out_dram = nc.dram_tensor("out", [T, F], F32, kind="ExternalOutput")

    # Rotated prefix-sum stationary: col 0 all ones, col t>=1 strict-upper.
    lt_np = np.triu(np.ones((P, P), dtype=np.float32), 1)
    lt_np[:, 0] = 1.0
    # recip[p, i]: psum partition p of block i holds output row
    #   r = i*128 + 127   (p == 0)
    #   r = i*128 + p - 1  (p >= 1)
    # scale is 1 / (r + 1).
    rows = np.arange(T, dtype=np.int64).reshape(NBLK, P)  # [i, p'] -> i*128+p'
    r_of_p = np.empty((NBLK, P), dtype=np.int64)
    r_of_p[:, 0] = rows[:, P - 1]
    r_of_p[:, 1:] = rows[:, : P - 1]
    recip_np = np.ascontiguousarray(
        (1.0 / (r_of_p.T + 1.0)).astype(np.float32)
    )  # [P, NBLK]
    lt_dram = nc.inline_tensor(lt_np, "lt_const")
    recip_dram = nc.inline_tensor(recip_np, "recip_const")

    with tile.TileContext(nc) as tc:
        with (
            tc.tile_pool(name="const", bufs=1) as cpool,
            tc.tile_pool(name="xin", bufs=4) as xpool,
            tc.tile_pool(name="xout", bufs=4) as opool,
            tc.tile_pool(name="run", bufs=2) as rpool,
            tc.tile_pool(name="psum", bufs=3, space="PSUM") as ppool,
        ):
            lt = cpool.tile([P, P], F32)
            nc.sync.dma_start(lt[:], lt_dram[:])
            recip = cpool.tile([P, NBLK], F32)
            nc.sync.dma_start(recip[:], recip_dram[:])

            running = None
            CPG = 2  # blocks per input DMA (1 MiB transfers)
            for g in range(NBLK // CPG):
                # [256, F] dram rows -> [128p, 2c, F] (row = c*128 + p)
                src = x_dram[g * CPG * P : (g + 1) * CPG * P, :].rearrange(
                    "(c p) f -> p c f", p=P
                )
                xt = xpool.tile([P, CPG, F], F32)
                # SWDGE: spreads HBM->SBUF across all 16 SDMA engines.
                nc.gpsimd.dma_start(xt[:], src)

                for c in range(CPG):
                    i = g * CPG + c
                    if running is not None:
                        nc.vector.tensor_tensor(
                            xt[0:1, c, :],
                            xt[0:1, c, :],
                            running[:],
                            mybir.AluOpType.add,
                        )

                    ps = ppool.tile([P, F], F32)
                    for h in range(NHALF):
                        nc.tensor.matmul(
                            ps[:, h * FH : (h + 1) * FH],
                            lt[:],
                            xt[:, c, h * FH : (h + 1) * FH],
                            start=True,
                            stop=True,
                        )

                    if i < NBLK - 1:
                        new_running = rpool.tile([1, F], F32)
                        nc.vector.tensor_copy(new_running[:], ps[0:1, :])
                        running = new_running

                    ot = opool.tile([P, F], F32)
                    nc.vector.tensor_scalar(
                        ot[:], ps[:], recip[:, i : i + 1], None, mybir.AluOpType.mult
                    )
                    # Undo the rotation: partition 0 is the block's last row.
                    nc.sync.dma_start(
                        out_dram[i * P : (i + 1) * P - 1, :], ot[1:P, :]
                    )
                    nc.sync.dma_start(
                        out_dram[(i + 1) * P - 1 : (i + 1) * P, :], ot[0:1, :]
                    )

    nc.compile()
    return nc


_NC_CACHE = None
last_results = None  # BassKernelResults of the most recent run (for test harness)


def kernel(inputs: np.ndarray) -> np.ndarray:
    global _NC_CACHE, last_results
    if _NC_CACHE is None:
        _NC_CACHE = _build()
    nc = _NC_CACHE
    x = np.ascontiguousarray(np.asarray(inputs, dtype=np.float32))
    assert x.shape == (B, T, F), x.shape
    in_maps = [{"x": x[b]} for b in range(B)]
    res = run_bass_kernel_spmd(nc, in_maps, core_ids=list(range(B)))
    last_results = res
    return np.stack([r["out"] for r in res.results], axis=0)


# revision 6
# speedup vs baseline: 2.1042x; 2.0999x over previous
"""Cumulative mean along T (running mean) for input [8, 4096, 1024] f32.

out[b, t, f] = mean(x[b, :t+1, f])

Pure data parallel over batch: 8 cores, one batch element each.
Per core, blocked prefix-sum along T in 128-row blocks with a rotated-output
triangular matmul:

  stationary lt[s, t]: col 0 = all ones; col t>=1 has ones for s <= t-1.
  With the running carry added into row 0 of the block beforehand,
    psum[0]    = carry + block total      (= output row 127 of the block)
    psum[t>=1] = carry + prefix(t-1)      (= output row t-1 of the block)
  so the next block's carry is read from PSUM partition 0 (legal AP base),
  and the output rows are just rotated by one partition, which the output
  DMA undoes. Per-row 1/(t+1) scaling via tensor_scalar with a per-partition
  reciprocal column (also rotated). All math in exact fp32.
"""

import numpy as np

import concourse.bacc as bacc
import concourse.tile as tile
from concourse import mybir
from concourse.bass_utils import run_bass_kernel_spmd

B, T, F = 8, 4096, 1024
P = 128
NBLK = T // P  # 32
FH = 512       # one PSUM bank of f32
NHALF = F // FH

F32 = mybir.dt.float32


def _build():
    nc = bacc.Bacc(None, target_bir_lowering=False)
    x_dram = nc.dram_tensor("x", [T, F], F32, kind="ExternalInput")
    out_dram = nc.dram_tensor("out", [T, F], F32, kind="ExternalOutput")

    # Rotated prefix-sum stationary: col 0 all ones, col t>=1 strict-upper.
    lt_np = np.triu(np.ones((P, P), dtype=np.float32), 1)
    lt_np[:, 0] = 1.0
    # recip[p, i]: psum partition p of block i holds output row
    #   r = i*128 + 127   (p == 0)
    #   r = i*128 + p - 1  (p >= 1)
    # scale is 1 / (r + 1).
    rows = np.arange(T, dtype=np.int64).reshape(NBLK, P)  # [i, p'] -> i*128+p'
    r_of_p = np.empty((NBLK, P), dtype=np.int64)
    r_of_p[:, 0] = rows[:, P - 1]
    r_of_p[:, 1:] = rows[:, : P - 1]
    recip_np = np.ascontiguousarray(
        (1.0 / (r_of_p.T + 1.0)).astype(np.float32)
    )  # [P, NBLK]
    lt_dram = nc.inline_tensor(lt_np, "lt_const")
    recip_dram = nc.inline_tensor(recip_np, "recip_const")

    with tile.TileContext(nc) as tc:
        with (
            tc.tile_pool(name="const", bufs=1) as cpool,
            tc.tile_pool(name="xin", bufs=4) as xpool,
            tc.tile_pool(name="xout", bufs=4) as opool,
            tc.tile_pool(name="run", bufs=2) as rpool,
            tc.tile_pool(name="psum", bufs=3, space="PSUM") as ppool,
        ):
            lt = cpool.tile([P, P], F32)
            nc.sync.dma_start(lt[:], lt_dram[:])
            recip = cpool.tile([P, NBLK], F32)
            nc.sync.dma_start(recip[:], recip_dram[:])

            running = None
            CPG = 2  # blocks per input DMA (1 MiB transfers)
            for g in range(NBLK // CPG):
                # [256, F] dram rows -> [128p, 2c, F] (row = c*128 + p)
                src = x_dram[g * CPG * P : (g + 1) * CPG * P, :].rearrange(
                    "(c p) f -> p c f", p=P
                )
                xt = xpool.tile([P, CPG, F], F32)
                # SWDGE: spreads HBM->SBUF across all 16 SDMA engines.
                nc.gpsimd.dma_start(xt[:], src)

                ot = opool.tile([P, CPG, F], F32)
                for c in range(CPG):
                    i = g * CPG + c
                    if running is not None:
                        nc.vector.tensor_tensor(
                            xt[0:1, c, :],
                            xt[0:1, c, :],
                            running[:],
                            mybir.AluOpType.add,
                        )

                    ps = ppool.tile([P, F], F32)
                    for h in range(NHALF):
                        nc.tensor.matmul(
                            ps[:, h * FH : (h + 1) * FH],
                            lt[:],
                            xt[:, c, h * FH : (h + 1) * FH],
                            start=True,
                            stop=True,
                        )

                    if i < NBLK - 1:
                        new_running = rpool.tile([1, F], F32)
                        nc.vector.tensor_copy(new_running[:], ps[0:1, :])
                        running = new_running

                    nc.vector.tensor_scalar(
                        ot[:, c, :],
                        ps[:],
                        recip[:, i : i + 1],
                        None,
                        mybir.AluOpType.mult,
                    )
                # Undo the rotation: partition 0 holds each block's last row.
                # dst rows g*256 + c*128 + p' <- ot[p'+1, c, :]
                dst_a = out_dram.rearrange("(n p) f -> p n f", p=P)[
                    0 : P - 1, g * CPG : (g + 1) * CPG, :
                ]
                nc.gpsimd.dma_start(dst_a, ot[1:P, :, :])
                # dst rows g*256 + c*128 + 127 <- ot[0, c, :]
                dst_b = out_dram.rearrange("(n p) f -> p n f", p=P)[
                    P - 1 : P, g * CPG : (g + 1) * CPG, :
                ]
                nc.gpsimd.dma_start(dst_b, ot[0:1, :, :])

    nc.compile()
    return nc


_NC_CACHE = None
last_results = None  # BassKernelResults of the most recent run (for test harness)


def kernel(inputs: np.ndarray) -> np.ndarray:
    global _NC_CACHE, last_results
    if _NC_CACHE is None:
        _NC_CACHE = _build()
    nc = _NC_CACHE
    x = np.ascontiguousarray(np.asarray(inputs, dtype=np.float32))
    assert x.shape == (B, T, F), x.shape
    in_maps = [{"x": x[b]} for b in range(B)]
    res = run_bass_kernel_spmd(nc, in_maps, core_ids=list(range(B)))
    last_results = res
    return np.stack([r["out"] for r in res.results], axis=0)
